# revision 1
# baseline (speedup 1.0000x reference)
"""Trainium2 Bass kernel for a bare KAN layer (PCHIP spline mixing).

Math: out[b, o] = sum_d f_{o,d}(x[b,d]) + bias[o], where f_{o,d} is the PCHIP
cubic interpolant of coeffs[o,d,:] on K=64 uniform knots over [-2, 2], with
linear extrapolation outside.

Device strategy (per core, data-parallel over batch):
  Any C^1 piecewise cubic on the knot grid is exactly
      f(t) = sum_k C[k]*alpha_k(t) + (h*S[k])*beta_k(t)
  in the Hermite cardinal basis
      alpha_k = w^2*(3-2w),  beta_k = (t-k)*w^2,  w = relu(1 - |t-k|)
  with t = (x-X_MIN)/h clamped to [0, K-1]; linear extrapolation outside the
  domain equals an extra  min(t,0)*hS[0] + max(t-(K-1),0)*hS[K-1]  term.

  Per input-dim d the kernel builds three [2K=128, B] fp16 "weight fields"
  w^2, w^3, w^2*(1+y) (w from two ACT relu passes off a PE-broadcast t) and
  contracts them in PSUM against fp16 tables 3C-hS / -2C / hS:
      w2*(3C-hS) + w3*(-2C) + w2*(1+y)*hS = C*alpha + hS*beta.
  The t-broadcast is an fp16 hi/lo split matmul (exact to ~1e-5).

Self-contained: hardcodes shapes B=8192, D=64, K=64, O=64, 8 cores.
"""

import sys

import numpy as np

sys.path.insert(0, "/opt/trn_rl_repo")

from concourse import bass, mybir  # noqa: E402
from concourse.bass_utils import run_bass_kernel_spmd  # noqa: E402
from concourse.tile import TileContext  # noqa: E402

F32 = mybir.dt.float32
F16 = mybir.dt.float16
ALU = mybir.AluOpType
AF = mybir.ActivationFunctionType

B, D, K, O = 8192, 64, 64, 64
NCORES = 8
BSH = B // NCORES          # 1024 batch rows per core
NCHUNK = 2                 # 512-column matmul chunks
CHUNK = BSH // NCHUNK      # 512
NPAIR = D // 2             # 32 d-pairs; each pair-tile has 128 = 2*64 rows
X_MIN, X_MAX = -2.0, 2.0
H = (X_MAX - X_MIN) / (K - 1)

# fp16 table tensor [128, CF16], interleaved per group:
# cols g*3*O + [0:O)=3C-hS, [O:2O)=-2C, [2O:3O)=hS
C16_T1 = 0
C16_T2 = O
C16_T3 = 2 * O
CF16 = NPAIR * 3 * O                    # 6144
GSPLIT = 8                              # groups in the first table DMA
# fp16 selector tensor [128, CS16]: bsel groups then etab
CS_BSEL = 0
CS_ETAB = CS_BSEL + NPAIR * 128         # 4096
CS16 = CS_ETAB + O                      # 4160

# fp32 const tensor [128, CF32]
C32_PK1 = 0                             # 1 + k
C32_MK1 = 1                             # 1 - k
C32_BIAS = 2
CF32 = 3

TRACE = False
LAST_EXEC_NS = None


def _pchip_slopes_uniform(y, h):
    """numpy float32 port of reference._pchip_slopes_uniform. y: [..., K]."""
    y = y.astype(np.float32)
    delta = ((y[..., 1:] - y[..., :-1]) / np.float32(h)).astype(np.float32)
    dp, dn = delta[..., :-1], delta[..., 1:]
    same_sign = dp * dn > 0
    d_mid = np.where(
        same_sign, (2.0 * dp * dn / (dp + dn + np.float32(1e-12))), np.float32(0.0)
    ).astype(np.float32)

    def _fix_endpoint(d_end, delta0, delta1):
        d_end = np.where(d_end * delta0 <= 0, np.float32(0.0), d_end)
        d_end = np.where(
            (delta0 * delta1 < 0) & (np.abs(d_end) > 3.0 * np.abs(delta0)),
            (3.0 * delta0).astype(np.float32),
            d_end,
        )
        return d_end.astype(np.float32)

    d0 = _fix_endpoint(
        ((3.0 * delta[..., 0] - delta[..., 1]) / 2.0).astype(np.float32),
        delta[..., 0],
        delta[..., 1],
    )
    dN = _fix_endpoint(
        ((3.0 * delta[..., -1] - delta[..., -2]) / 2.0).astype(np.float32),
        delta[..., -1],
        delta[..., -2],
    )
    return np.concatenate([d0[..., None], d_mid, dN[..., None]], axis=-1)


def _build_kernel(repeat=1):
    nc = bass.Bass()

    xt = nc.declare_dram_parameter("xt", [D, BSH], F32, isOutput=False)
    c16 = nc.declare_dram_parameter("c16", [128, CF16], F16, isOutput=False)
    cs16 = nc.declare_dram_parameter("cs16", [128, CS16], F16, isOutput=False)
    c32 = nc.declare_dram_parameter("c32", [128, CF32], F32, isOutput=False)
    outt = nc.declare_dram_parameter("outt", [O, BSH], F32, isOutput=True)

    with TileContext(nc) as tc:
        with (
            tc.tile_pool(name="consts", bufs=1) as consts,
            tc.tile_pool(name="work", bufs=6) as work,
            tc.tile_pool(name="tcbp", bufs=3, space="PSUM") as tcbp,
            tc.tile_pool(name="accp", bufs=1, space="PSUM") as accp,
        ):
            xt_sb = consts.tile([D, BSH], F32)
            c16_sb = consts.tile([128, CF16], F16)
            cs16_sb = consts.tile([128, CS16], F16)
            c32_sb = consts.tile([128, CF32], F32)
            nc.sync.dma_start(xt_sb[:], xt[:])
            nc.sync.dma_start(cs16_sb[:], cs16[:])
            nc.sync.dma_start(
                c16_sb[:, : GSPLIT * 3 * O], c16[:, : GSPLIT * 3 * O]
            )
            nc.sync.dma_start(
                c16_sb[:, GSPLIT * 3 * O :], c16[:, GSPLIT * 3 * O :]
            )
            nc.sync.dma_start(c32_sb[:], c32[:])

            def tab(base, g):
                lo = g * 3 * O + base
                return c16_sb[:, lo : lo + O]

            etab_t = cs16_sb[:, CS_ETAB : CS_ETAB + O]

            def bsel_t(g):
                return cs16_sb[:, CS_BSEL + g * 128 : CS_BSEL + (g + 1) * 128]

            pk1_t = c32_sb[:, C32_PK1 : C32_PK1 + 1]
            mk1_t = c32_sb[:, C32_MK1 : C32_MK1 + 1]
            bias_t = c32_sb[0:O, C32_BIAS : C32_BIAS + 1]

            # PSUM accumulator [O, 1024] (2 banks); its first use doubles as
            # the warm matmul that absorbs the c16 DMA semaphore into PE's
            # clock (walrus allows only one sync wait per instruction).
            acc = accp.tile([O, NCHUNK * CHUNK], F32)
            nc.tensor.matmul(
                acc[0:64, 0:128],
                cs16_sb[:, 0:64],
                cs16_sb[:, 0:128],
                start=True,
                stop=True,
            )
            nc.tensor.matmul(
                acc[0:64, 0:128],
                c16_sb[:, 0:64],
                c16_sb[:, 0:128],
                start=True,
                stop=True,
            )
            nc.tensor.matmul(
                acc[0:64, 0:128],
                c16_sb[:, GSPLIT * 3 * O : GSPLIT * 3 * O + 64],
                c16_sb[:, GSPLIT * 3 * O : GSPLIT * 3 * O + 128],
                start=True,
                stop=True,
            )
            # absorb input DMAs into DVE + ACT clocks
            ab = consts.tile([128, 2], F32, tag="absorb")
            nc.vector.tensor_copy(ab[:, 0:1], c32_sb[:, 0:1])
            nc.vector.tensor_copy(ab[0:D, 1:2], xt_sb[:, 0:1])
            nc.scalar.copy(ab[0:D, 1:2], xt_sb[:, 0:1])
            nc.scalar.copy(ab[:, 0:1], c32_sb[:, 0:1])

            # t = (x + 2) * (1/h); tc = clamp(t, 0, K-1); fp16 hi/lo split
            t_sb = consts.tile([D, BSH], F32)
            tc_sb = consts.tile([D, BSH], F32)
            nc.vector.tensor_scalar(
                t_sb[:], xt_sb[:], float(-X_MIN), float(1.0 / H), ALU.add, ALU.mult
            )
            nc.vector.tensor_scalar(
                tc_sb[:], t_sb[:], 0.0, float(K - 1), ALU.max, ALU.min
            )
            tc2 = consts.tile([128, BSH], F16)        # rows 0:64 hi, 64:128 lo
            tlo = consts.tile([D, BSH], F32)
            nc.vector.tensor_copy(tc2[0:D, :], tc_sb[:])
            nc.vector.tensor_tensor(tlo[:], tc_sb[:], tc2[0:D, :], ALU.subtract)
            nc.vector.tensor_copy(tc2[D:128, :], tlo[:])

            # Edge (extrapolation) fields, one per chunk:
            # rows 0:64 = min(t,0) -> hS[d,0]; rows 64:128 = max(t-63,0) -> hS[d,63]
            edges = []
            for c in range(NCHUNK):
                e = consts.tile([128, CHUNK], F16, tag=f"edge{c}")
                rows = t_sb[:, c * CHUNK : (c + 1) * CHUNK]
                nc.vector.tensor_scalar(e[0:64, :], rows, 0.0, None, ALU.min)
                nc.vector.tensor_scalar(
                    e[64:128, :], rows, float(-(K - 1)), 0.0, ALU.add, ALU.max
                )
                edges.append(e)

            ob_full = consts.tile([O, BSH], F32)

            for _rep in range(max(1, repeat)):
                for g in range(NPAIR):
                    # broadcast t of (d0,d1)=(2g,2g+1): hi+lo fp16 split matmul
                    tcb = tcbp.tile([128, NCHUNK * CHUNK], F32, tag="tcb")
                    for c in range(NCHUNK):
                        nc.tensor.matmul(
                            tcb[:, c * CHUNK : (c + 1) * CHUNK],
                            bsel_t(g),
                            tc2[:, c * CHUNK : (c + 1) * CHUNK],
                            start=True,
                            stop=True,
                        )
                    # hat half-fields: ap = relu(1-y), bp = relu(1+y), y = t-k
                    bp_ = work.tile([128, NCHUNK * CHUNK], F16, tag="bp_")
                    nc.scalar.activation(
                        bp_[:], tcb[:], AF.Relu, bias=mk1_t, scale=1.0
                    )
                    w = work.tile([128, NCHUNK * CHUNK], F16, tag="w")
                    if g % 4 == 1:
                        # DVE-only hat: w = max(min(2-bp, bp), 0)
                        r_ = work.tile([128, NCHUNK * CHUNK], F16, tag="r_")
                        nc.vector.tensor_scalar(
                            r_[:], bp_[:], -1.0, 2.0, ALU.mult, ALU.add
                        )
                        m1 = work.tile([128, NCHUNK * CHUNK], F16, tag="m1")
                        nc.vector.tensor_tensor(m1[:], r_[:], bp_[:], ALU.min)
                        nc.vector.tensor_scalar(w[:], m1[:], 0.0, None, ALU.max)
                    else:
                        ap_ = work.tile([128, NCHUNK * CHUNK], F16, tag="ap_")
                        nc.scalar.activation(
                            ap_[:], tcb[:], AF.Relu, bias=pk1_t, scale=-1.0
                        )
                        nc.vector.tensor_tensor(w[:], ap_[:], bp_[:], ALU.min)
                    w2 = work.tile([128, NCHUNK * CHUNK], F16, tag="w2")
                    nc.vector.tensor_tensor(w2[:], w[:], w[:], ALU.mult)
                    w3 = work.tile([128, NCHUNK * CHUNK], F16, tag="w3")
                    nc.vector.tensor_tensor(w3[:], w2[:], w[:], ALU.mult)
                    wb = work.tile([128, NCHUNK * CHUNK], F16, tag="wb")
                    if g % 8 == 7:
                        nc.vector.tensor_tensor(wb[:], w2[:], bp_[:], ALU.mult)
                    else:
                        nc.gpsimd.tensor_tensor(wb[:], w2[:], bp_[:], ALU.mult)
                    for c in range(NCHUNK):
                        sl = slice(c * CHUNK, (c + 1) * CHUNK)
                        out_sl = acc[:, sl]
                        nc.tensor.matmul(
                            out_sl, tab(C16_T1, g), w2[:, sl],
                            start=(g == 0), stop=False,
                        )
                        nc.tensor.matmul(
                            out_sl, tab(C16_T2, g), w3[:, sl],
                            start=False, stop=False,
                        )
                        nc.tensor.matmul(
                            out_sl, tab(C16_T3, g), wb[:, sl],
                            start=False, stop=False,
                        )
                for c in range(NCHUNK):
                    out_sl = acc[:, c * CHUNK : (c + 1) * CHUNK]
                    nc.tensor.matmul(
                        out_sl, etab_t, edges[c][:], start=False, stop=True
                    )
                    nc.vector.tensor_scalar(
                        ob_full[:, c * CHUNK : (c + 1) * CHUNK], out_sl, bias_t,
                        None, ALU.add,
                    )
                nc.sync.dma_start(outt[:], ob_full[:])

    _split_multiwaits(nc)
    return nc


def _split_multiwaits(nc):
    """walrus (neuronx-cc) allows one sync wait per instruction; move extra
    waits onto standalone NoOps inserted just before the offender."""
    cnt = 0
    for f in nc.m.functions:
        for blk in f.blocks:
            out = []
            changed = False
            for ins in blk.instructions:
                si = ins.sync_info
                if si is not None and len(si.on_wait) > 1:
                    waits = list(si.on_wait)
                    for w in waits[:-1]:
                        nop = mybir.InstNoOp(name=f"I-ws-{cnt}", ins=[], outs=[])
                        cnt += 1
                        nop.engine = ins.engine
                        nop.sync_info = type(si)(on_wait=[w], on_update=[])
                        out.append(nop)
                    ins.sync_info = type(si)(
                        on_wait=[waits[-1]], on_update=list(si.on_update)
                    )
                    changed = True
                out.append(ins)
            if changed:
                blk.instructions = out


def _host_tables(coeffs, bias):
    coeffs = np.ascontiguousarray(np.asarray(coeffs, dtype=np.float32))
    bias = np.asarray(bias, dtype=np.float32)
    slopes = _pchip_slopes_uniform(coeffs, H)          # [O, D, K]
    hs = (slopes * np.float32(H)).astype(np.float32)   # h * S

    ct = coeffs.transpose(1, 2, 0)                     # [D, K, O]
    st = hs.transpose(1, 2, 0)                         # [D, K, O]

    def pairs(a):                                      # [D,K,O] -> [128, 32*O]
        return np.ascontiguousarray(
            a.reshape(NPAIR, 2 * K, O).transpose(1, 0, 2).reshape(128, NPAIR * O)
        )

    c16 = np.zeros((128, CF16), dtype=np.float16)
    p1, p2, p3 = pairs(3.0 * ct - st), pairs(-2.0 * ct), pairs(st)
    for g in range(NPAIR):
        lo = g * 3 * O
        c16[:, lo : lo + O] = p1[:, g * O : (g + 1) * O]
        c16[:, lo + O : lo + 2 * O] = p2[:, g * O : (g + 1) * O]
        c16[:, lo + 2 * O : lo + 3 * O] = p3[:, g * O : (g + 1) * O]
    cs16 = np.zeros((128, CS16), dtype=np.float16)
    cs16[0:64, CS_ETAB : CS_ETAB + O] = st[:, 0, :]
    cs16[64:128, CS_ETAB : CS_ETAB + O] = st[:, K - 1, :]
    for g in range(NPAIR):
        base = CS_BSEL + g * 128
        cs16[2 * g, base : base + 64] = 1.0            # hi row d0 -> parts 0:64
        cs16[2 * g + 1, base + 64 : base + 128] = 1.0  # hi row d1 -> parts 64:128
        cs16[64 + 2 * g, base : base + 64] = 1.0       # lo row d0
        cs16[64 + 2 * g + 1, base + 64 : base + 128] = 1.0

    c32 = np.zeros((128, CF32), dtype=np.float32)
    kk = np.arange(128, dtype=np.float32) % K
    c32[:, C32_PK1] = 1.0 + kk
    c32[:, C32_MK1] = 1.0 - kk
    c32[0:O, C32_BIAS] = bias
    return c16, cs16, c32


def kernel(x, coeffs, bias):
    global LAST_EXEC_NS
    x = np.asarray(x, dtype=np.float32)
    c16, cs16, c32 = _host_tables(coeffs, bias)

    in_maps = []
    for r in range(NCORES):
        xc = x[r * BSH : (r + 1) * BSH, :]             # [1024, 64]
        in_maps.append(
            {"xt": np.ascontiguousarray(xc.T), "c16": c16, "cs16": cs16, "c32": c32}
        )

    nc = _build_kernel()
    res = run_bass_kernel_spmd(nc, in_maps, list(range(NCORES)), trace=TRACE)
    LAST_EXEC_NS = getattr(res, "exec_time_ns", None)

    out = np.empty((B, O), dtype=np.float32)
    for r in range(NCORES):
        out_t = res.results[r]["outt"]                 # [O, 1024]
        out[r * BSH : (r + 1) * BSH, :] = np.asarray(out_t).T
    return out


if __name__ == "__main__":
    rng = np.random.default_rng(0)
    x = rng.standard_normal((B, D)).astype(np.float32)
    coeffs = (0.01 * rng.standard_normal((O, D, K))).astype(np.float32)
    bias = np.zeros((O,), dtype=np.float32)
    out = kernel(x, coeffs, bias)
    print("out", out.shape, out.dtype, float(np.abs(out).mean()))

